# revision 26
# baseline (speedup 1.0000x reference)
"""AttentionBlock (GroupNorm -> QKV -> 8-head attention -> proj -> residual)
as a Bass/Tile kernel for Trainium2, data-parallel over batch on 8 cores.

Self-contained: hardcodes shapes B=8, C=512, H=W=32 (N=1024), heads=8, d=64,
groups=32.  Each core processes one batch element; all params replicated.

v2 design (engine-overlap rewrite of the serial-phase baseline):
  * all-bf16 dataflow: x, weights, activations are bf16 (PE streams 1
    cycle/col for any free size; DMA traffic halves; f32 only in PSUM
    accumulators and GN statistics).
  * pair-blocked QKV: weights laid out per head-pair (q|k|v m-tiles of 128
    rows = 2 heads x 64); score matmuls contract over K=64 partition rows
    directly (no zero-padded K copy), saving 4 of 16 QKV m-tiles.
  * softmax denominators via the ones-block trick (vT cols 64-127 are ones so
    context rows 64-127 accumulate sum(probs) for free), then DVE
    reciprocal_approx_fast + multiply -- the ACT engine runs ONLY exp (one
    table load), never switching LUTs mid-kernel.
  * software pipeline: pair j's score matmuls / exp / context accumulation
    run with QKV + v-transposes of pair j+1 interleaved in the PE stream as
    filler, so the PE never waits on the ACT exp backlog and the HAM
    clock-gate stays warm (PE p-state ramps 0.65->2.4 GHz over ~3us).
  * context accumulated per query-half ([128,512] PSUM tiles) so the PSUM
    budget fits exactly 8 banks: scores 2x2 + context 2x1 + qkv/transpose
    staging 2x1.  Pair 3 (no qkv filler left) interleaves its second-half
    context into the kt loop, borrowing the idle staging banks.
  * v-transposes batched: 8 PE transposes land in one [128,512] bf16 PSUM
    tile, evacuated by a single strided DVE copy into persistent vT tiles
    whose ones-columns are memset once.
"""

import sys

sys.path.insert(0, "/opt/trn_rl_repo")

import numpy as np
import ml_dtypes

B, C, HH, WW = 8, 512, 32, 32
N = HH * WW          # 1024
NH, HD = 8, 64       # heads, head dim
NG = 32              # groupnorm groups
EPS = 1e-5
NT = C // 128        # 4 channel tiles
KT = N // 128        # 8 key tiles
NP = NH // 2         # 4 head pairs
NCORES = 8
LAG = 2

_CACHE: dict = {}
DEBUG = False


def _build_program():
    import concourse.bacc as bacc
    import concourse.tile as tile
    from concourse import mybir

    f32 = mybir.dt.float32
    bf16 = mybir.dt.bfloat16
    AF = mybir.ActivationFunctionType
    OP = mybir.AluOpType

    nc = bacc.Bacc("TRN2", target_bir_lowering=False, debug=False)

    x_d = nc.dram_tensor("x", [C, N], bf16, kind="ExternalInput").ap()
    # pair-blocked qkv weights: [pair, ktile, q|k|v, cin 128, cout 128]
    wq_d = nc.dram_tensor("wqkvT", [NP, NT, 3, 128, 128], bf16,
                          kind="ExternalInput").ap()
    wp_d = nc.dram_tensor("wprojT", [NT, 128, C], bf16, kind="ExternalInput").ap()
    smalls_d = nc.dram_tensor("smalls", [128, 32], f32, kind="ExternalInput").ap()
    gmaskT_d = nc.dram_tensor("gmaskT", [8, 128], f32, kind="ExternalInput").ap()
    ident_d = nc.dram_tensor("ident2", [128, 128], bf16, kind="ExternalInput").ap()
    out_d = nc.dram_tensor("out", [C, N], bf16, kind="ExternalOutput").ap()

    x_dt = x_d.rearrange("(t p) n -> t p n", p=128)
    out_dt = out_d.rearrange("(t p) n -> t p n", p=128)

    from contextlib import ExitStack

    with tile.TileContext(nc) as tc, ExitStack() as ctx:
        sg = ctx.enter_context(tc.tile_pool(name="sg", bufs=1))
        work = ctx.enter_context(tc.tile_pool(name="work", bufs=1))
        pb_pool = ctx.enter_context(tc.tile_pool(name="pbp", bufs=2))
        small = ctx.enter_context(tc.tile_pool(name="small", bufs=4))
        outp = ctx.enter_context(tc.tile_pool(name="outp", bufs=4))
        rsp = ctx.enter_context(tc.tile_pool(name="rsp", bufs=4))
        # PSUM (8 banks): pA = scores 2x[128,1024]f32 (2 banks each),
        # pX = context accumulators 2x[128,512]f32, pC = staging 2x[128,512]
        # (qkv halves / transpose batches / GN / proj; pair 3 borrows it for
        # its second-half context).
        pA = ctx.enter_context(tc.tile_pool(name="pA", bufs=2, space="PSUM"))
        pX = ctx.enter_context(tc.tile_pool(name="pX", bufs=2, space="PSUM"))
        pC = ctx.enter_context(tc.tile_pool(name="pC", bufs=2, space="PSUM"))

        # ---- input DMAs ----
        x_sb = []
        for t in range(NT):
            xt = work.tile([128, N], bf16, name=f"x{t}", tag=f"x{t}")
            nc.sync.dma_start(out=xt[:, 0:512], in_=x_dt[t][:, 0:512])
            nc.sync.dma_start(out=xt[:, 512:1024], in_=x_dt[t][:, 512:1024])
            x_sb.append(xt)

        # pair-0 qkv weights first (first consumer), then packed small
        # params, then the remaining weight blocks.
        w_sb = []
        for j in range(NP):
            w_sb.append(
                sg.tile([128, NT * 3 * 128], bf16, name=f"w{j}", tag=f"w{j}")
            )

        def dma_wpair(j):
            nc.sync.dma_start(
                out=w_sb[j].rearrange("p (b c) -> p b c", c=128),
                in_=wq_d[j].rearrange("k t p c -> p (k t) c"),
            )

        dma_wpair(0)
        smalls_sb = sg.tile([128, 32], f32, name="smalls_sb")
        nc.sync.dma_start(out=smalls_sb, in_=smalls_d)
        bqkv_sb = smalls_sb[:, 0:12]
        bproj_sb = smalls_sb[:, 12:16]
        gnw_sb = smalls_sb[:, 16:20]
        gnb_sb = smalls_sb[:, 20:24]
        gmask_sb = smalls_sb[:, 24:32]
        gmaskT_sb = sg.tile([8, 128], f32, name="gmaskT_sb")
        nc.sync.dma_start(out=gmaskT_sb, in_=gmaskT_d)
        ident_sb = sg.tile([128, 128], bf16, name="ident_sb")
        nc.sync.dma_start(out=ident_sb, in_=ident_d)
        for j in range(1, NP):
            dma_wpair(j)
        wp_all = sg.tile([128, NT * C], bf16, name="wp_all")
        nc.sync.dma_start(
            out=wp_all.rearrange("p (t c) -> p t c", c=C),
            in_=wp_d.rearrange("t p c -> p t c"),
        )
        wp_sb = [wp_all[:, t * C : (t + 1) * C] for t in range(NT)]

        eps_sb = sg.tile([8, 1], f32, name="eps_sb")
        nc.gpsimd.memset(eps_sb, EPS)
        # dummy activation: pulls the Sqrt ACT-table load to t~7us (the
        # implicit load otherwise inherits the real sqrt's waits and lands
        # on the GroupNorm critical path)
        sqrt_warm = sg.tile([8, 1], f32, name="sqrt_warm")
        nc.scalar.activation(out=sqrt_warm, in_=eps_sb, func=AF.Sqrt, scale=1.0)

        # persistent vT tiles (per kt a [64 ones | 64 v] block; ones memset
        # once) and zero-padded per-head K tiles (K=64-contraction matmuls
        # misbehave on HW; keep full-128 contraction with zeros in the other
        # head's rows).  Memsets run on the idle GPSIMD engine, ordered so
        # pair 0's tiles (first consumers) are ready earliest.
        vt_sb = [[None, None] for _ in range(NP)]
        kp_sb = [[None, None] for _ in range(NP)]
        for j in range(NP):
            for h01 in range(2):
                kp = sg.tile([128, N], bf16, name=f"kp{j}_{h01}")
                po = (1 - h01) * HD
                nc.gpsimd.memset(kp[po : po + HD, :], 0.0)
                kp_sb[j][h01] = kp
            for h01 in range(2):
                vt = sg.tile([128, N], bf16, name=f"vt{j}_{h01}")
                nc.gpsimd.memset(
                    vt.rearrange("p (k c) -> p k c", c=128)[:, :, 0:HD], 1.0
                )
                vt_sb[j][h01] = vt

        # ---- GroupNorm statistics ----
        allstats = sg.tile([128, 2 * NT], f32, name="allstats")
        for t in range(NT):
            bns = small.tile([128, 2, 6], f32, name=f"bns{t}", tag="bns")
            nc.vector.bn_stats(out=bns[:, 0, :], in_=x_sb[t][:, 0:512])
            nc.vector.bn_stats(out=bns[:, 1, :], in_=x_sb[t][:, 512:1024])
            nc.vector.bn_aggr(out=allstats[:, 2 * t : 2 * t + 2], in_=bns)
            m2 = small.tile([128, 1], f32, name=f"m2_{t}", tag="m2")
            nc.vector.tensor_mul(
                m2, allstats[:, 2 * t : 2 * t + 1], allstats[:, 2 * t : 2 * t + 1]
            )
            nc.vector.tensor_add(
                allstats[:, 2 * t + 1 : 2 * t + 2],
                allstats[:, 2 * t + 1 : 2 * t + 2],
                m2,
            )

        grp_ps = pC.tile([8, 2 * NT], f32, name="grp_ps", tag="acc")
        nc.tensor.matmul(grp_ps, gmask_sb, allstats)
        grp_sb = sg.tile([8, 2 * NT], f32, name="grp_sb")
        nc.vector.tensor_copy(grp_sb, grp_ps)
        msq = sg.tile([8, NT], f32, name="msq")
        nc.vector.tensor_mul(msq, grp_sb[:, 0 : 2 * NT : 2], grp_sb[:, 0 : 2 * NT : 2])
        nc.vector.tensor_sub(
            grp_sb[:, 1 : 2 * NT : 2], grp_sb[:, 1 : 2 * NT : 2], msq
        )
        nc.scalar.activation(
            out=grp_sb[:, 1 : 2 * NT : 2],
            in_=grp_sb[:, 1 : 2 * NT : 2],
            func=AF.Sqrt,
            bias=eps_sb,
            scale=1.0,
        )
        nc.vector.reciprocal(grp_sb[:, 1 : 2 * NT : 2], grp_sb[:, 1 : 2 * NT : 2])

        chan_ps = pC.tile([128, 2 * NT], f32, name="chan_ps", tag="acc")
        nc.tensor.matmul(chan_ps, gmaskT_sb, grp_sb)
        chan_sb = sg.tile([128, 2 * NT], f32, name="chan_sb")
        nc.vector.tensor_copy(chan_sb, chan_ps)

        A_sb = sg.tile([128, NT], f32, name="A_sb")
        nc.vector.tensor_mul(A_sb, chan_sb[:, 1 : 2 * NT : 2], gnw_sb)
        B_sb = sg.tile([128, NT], f32, name="B_sb")
        nc.vector.tensor_mul(B_sb, chan_sb[:, 0 : 2 * NT : 2], A_sb)
        nc.vector.tensor_sub(B_sb, gnb_sb, B_sb)

        xn_sb = []
        for t in range(NT):
            xn_sb.append(work.tile([128, N], bf16, name=f"xn{t}", tag=f"xn{t}"))
        for hlf in range(2):
            for t in range(NT):
                nc.vector.tensor_scalar(
                    out=xn_sb[t][:, hlf * 512 : (hlf + 1) * 512],
                    in0=x_sb[t][:, hlf * 512 : (hlf + 1) * 512],
                    scalar1=A_sb[:, t : t + 1],
                    scalar2=B_sb[:, t : t + 1],
                    op0=OP.mult,
                    op1=OP.add,
                )

        # ---- QKV / transpose building blocks ----
        q_sb = [None] * NP
        k_sb = [None] * NP
        v_sb = [None] * NP

        def qkv_half(j, ty, hlf):
            """One m-half of pair j's q/k/v projection (4 matmuls + evac)."""
            dest = (q_sb, k_sb, v_sb)[ty]
            if dest[j] is None:
                dest[j] = work.tile(
                    [128, N], bf16, name=f"qkv{j}_{ty}", tag=f"qkv{j}_{ty}"
                )
            acc = pC.tile([128, 512], f32, name=f"acc{j}_{ty}_{hlf}", tag="acc")
            for kc in range(NT):
                nc.tensor.matmul(
                    acc,
                    w_sb[j][:, (kc * 3 + ty) * 128 : (kc * 3 + ty + 1) * 128],
                    xn_sb[kc][:, hlf * 512 : (hlf + 1) * 512],
                    start=(kc == 0),
                    stop=(kc == NT - 1),
                )
            nc.vector.tensor_scalar_add(
                dest[j][:, hlf * 512 : (hlf + 1) * 512],
                acc,
                bqkv_sb[:, j * 3 + ty : j * 3 + ty + 1],
            )
            if ty == 1:
                # stage this K half into the zero-padded per-head tiles
                for h01 in range(2):
                    po = h01 * HD
                    nc.vector.tensor_copy(
                        kp_sb[j][h01][po : po + HD, hlf * 512 : (hlf + 1) * 512],
                        k_sb[j][po : po + HD, hlf * 512 : (hlf + 1) * 512],
                    )

        def vtrans_pair(j):
            """All 8 key-chunks of BOTH heads' v transposed in one go
            ([128,128] blocks), then per-head strided DVE copies into the
            persistent vT tiles' v-slots."""
            tp = pC.tile([128, N], bf16, name=f"tp{j}", tag="acc")
            for kt in range(KT):
                nc.tensor.transpose(
                    tp[:, kt * 128 : (kt + 1) * 128],
                    v_sb[j][:, kt * 128 : (kt + 1) * 128],
                    ident_sb,
                )
            for h01 in range(2):
                nc.vector.tensor_copy(
                    vt_sb[j][h01].rearrange("p (k c) -> p k c", c=128)[:, :, HD:128],
                    tp.rearrange("p (k c) -> p k c", c=128)[:, :, h01 * HD : (h01 + 1) * HD],
                )

        def qkv_pair(j):
            for ty in range(3):
                for hlf in range(2):
                    qkv_half(j, ty, hlf)

        hatt_sb = []
        for t in range(NT):
            ht = work.tile([128, N], bf16, name=f"hatt{t}", tag=f"hatt{t}")
            hatt_sb.append(ht)

        dbg_tiles = {}
        if DEBUG:
            dbg_tiles["pb00"] = sg.tile([128, N], bf16, name="dbg_t_pb00")
            dbg_tiles["cx10"] = sg.tile([128, 512], f32, name="dbg_t_cx10")
            dbg_tiles["rs00"] = sg.tile([HD, 512], f32, name="dbg_t_rs00")

        # ---- attention pair with interleaved filler ----
        def attn_pair(j):
            last = j == NP - 1
            pbs = {}
            cx1 = {}
            cx2 = {}
            # filler units for pairs 0-2: qkv halves + v transposes of j+1
            filler = []
            if not last:
                jn = j + 1
                for ty in range(3):
                    for hlf in range(2):
                        filler.append(lambda ty=ty, hlf=hlf: qkv_half(jn, ty, hlf))
                filler.append(lambda: vtrans_pair(jn))

            def emit_sc(kt):
                for h01 in range(2):
                    sc = pA.tile([128, N], f32, name=f"sc{h01}_{kt}", tag="sc")
                    for hlf in range(2):
                        nc.tensor.matmul(
                            sc[:, hlf * 512 : (hlf + 1) * 512],
                            kp_sb[j][h01][:, kt * 128 : (kt + 1) * 128],
                            q_sb[j][:, hlf * 512 : (hlf + 1) * 512],
                        )
                    pb = pb_pool.tile(
                        [128, N], bf16, name=f"pb{h01}_{kt}", tag=f"pb{h01}_{kt}"
                    )
                    nc.scalar.activation(out=pb, in_=sc, func=AF.Exp, scale=0.125)
                    pbs[(h01, kt)] = pb
                    if DEBUG and j == 0 and h01 == 0 and kt == 0:
                        nc.vector.tensor_copy(dbg_tiles["pb00"], pb)

            def emit_cx1(kt):
                for h01 in range(2):
                    if kt == 0:
                        cx1[h01] = pX.tile(
                            [128, 512], f32, name=f"cx1_{h01}", tag="cx"
                        )
                    nc.tensor.matmul(
                        cx1[h01],
                        vt_sb[j][h01][:, kt * 128 : (kt + 1) * 128],
                        pbs[(h01, kt)][:, 0:512],
                        start=(kt == 0),
                        stop=(kt == KT - 1),
                    )

            def emit_cx2(kt):
                for h01 in range(2):
                    if kt == 0:
                        pool, tag = (pC, "acc") if last else (pX, "cx")
                        cx2[h01] = pool.tile(
                            [128, 512], f32, name=f"cx2_{h01}", tag=tag
                        )
                    nc.tensor.matmul(
                        cx2[h01],
                        vt_sb[j][h01][:, kt * 128 : (kt + 1) * 128],
                        pbs[(h01, kt)][:, 512:1024],
                        start=(kt == 0),
                        stop=(kt == KT - 1),
                    )

            def normalize(cx, hlf):
                for h01 in range(2):
                    rsb = rsp.tile([HD, 512], f32, name=f"rs{h01}", tag="rs")
                    if DEBUG and j == 0 and h01 == 0 and hlf == 0:
                        nc.vector.tensor_copy(dbg_tiles["cx10"], cx[h01])
                    nc.vector.reciprocal_approx_fast(out=rsb, in_=cx[h01][0:HD, :])
                    if DEBUG and j == 0 and h01 == 0 and hlf == 0:
                        nc.vector.tensor_copy(dbg_tiles["rs00"], rsb)
                    nc.vector.tensor_mul(
                        hatt_sb[j][h01 * HD : (h01 + 1) * HD,
                                   hlf * 512 : (hlf + 1) * 512],
                        cx[h01][HD:128, :],
                        rsb,
                    )

            # kt loop: scores + exp lead; first-half context LAGs; filler
            # (next pair's qkv) keeps the PE fed while ACT drains the exp
            # backlog.  Pair 3 interleaves second-half context instead.
            fi = 0
            for kt in range(KT):
                emit_sc(kt)
                if kt >= LAG:
                    emit_cx1(kt - LAG)
                    if last:
                        emit_cx2(kt - LAG)
                if kt > 0 and fi < len(filler):
                    filler[fi]()
                    fi += 1
            for kt in range(KT - LAG, KT):
                emit_cx1(kt)
                if last:
                    emit_cx2(kt)
            while fi < len(filler):
                filler[fi]()
                fi += 1
            normalize(cx1, 0)
            if not last:
                for kt in range(KT):
                    emit_cx2(kt)
            normalize(cx2, 1)

        qkv_pair(0)
        vtrans_pair(0)
        for j in range(NP):
            attn_pair(j)

        # ---- proj + bias + residual ----
        for mt in range(NT):
            ot = outp.tile([128, N], bf16, name=f"ot{mt}", tag="ot")
            for hlf in range(2):
                # alternate pX/pC so four accumulators are in flight and
                # the STT evacuations never stall the PE (pX frees after
                # pair 3's first-half normalize; pC after its second).
                ppool, ptag = (pX, "cx") if hlf == 0 else (pC, "acc")
                pp = ppool.tile([128, 512], f32, name=f"pp{mt}_{hlf}", tag=ptag)
                for kc in range(NT):
                    nc.tensor.matmul(
                        pp,
                        wp_sb[kc][:, mt * 128 : (mt + 1) * 128],
                        hatt_sb[kc][:, hlf * 512 : (hlf + 1) * 512],
                        start=(kc == 0),
                        stop=(kc == NT - 1),
                    )
                nc.vector.scalar_tensor_tensor(
                    out=ot[:, hlf * 512 : (hlf + 1) * 512],
                    in0=pp,
                    scalar=bproj_sb[:, mt : mt + 1],
                    in1=x_sb[mt][:, hlf * 512 : (hlf + 1) * 512],
                    op0=OP.add,
                    op1=OP.add,
                )
                nc.sync.dma_start(
                    out=out_dt[mt][:, hlf * 512 : (hlf + 1) * 512],
                    in_=ot[:, hlf * 512 : (hlf + 1) * 512],
                )

        if DEBUG:
            dbg_specs = [
                ("dbg_w0", w_sb[0], [128, NT * 3 * 128], bf16),
                ("dbg_xn0", xn_sb[0], [128, N], bf16),
                ("dbg_q0", q_sb[0], [128, N], bf16),
                ("dbg_k0", k_sb[0], [128, N], bf16),
                ("dbg_kp00", kp_sb[0][0], [128, N], bf16),
                ("dbg_v0", v_sb[0], [128, N], bf16),
                ("dbg_vt00", vt_sb[0][0], [128, N], bf16),
                ("dbg_hatt0", hatt_sb[0], [128, N], bf16),
                ("dbg_A", A_sb, [128, NT], f32),
                ("dbg_pb00", dbg_tiles["pb00"], [128, N], bf16),
                ("dbg_cx10", dbg_tiles["cx10"], [128, 512], f32),
                ("dbg_rs00", dbg_tiles["rs00"], [HD, 512], f32),
            ]
            for nm, tile_, shp, dt_ in dbg_specs:
                d = nc.dram_tensor(nm, shp, dt_, kind="ExternalOutput").ap()
                nc.sync.dma_start(out=d, in_=tile_)

    nc.compile()
    return nc


def _get_nc():
    if "nc" not in _CACHE:
        _CACHE["nc"] = _build_program()
    return _CACHE["nc"]


def _host_inputs(x, gn_w, gn_b, qkv_w, qkv_b, proj_w, proj_b):
    f32 = np.float32
    bf = ml_dtypes.bfloat16
    x = np.asarray(x, dtype=f32).reshape(B, C, N)
    gn_w = np.asarray(gn_w, dtype=f32)
    gn_b = np.asarray(gn_b, dtype=f32)
    qkv_w = np.asarray(qkv_w, dtype=f32)
    qkv_b = np.asarray(qkv_b, dtype=f32)
    proj_w = np.asarray(proj_w, dtype=f32)
    proj_b = np.asarray(proj_b, dtype=f32)

    # pair-blocked qkv weights: [pair, ktile, q|k|v, cin 128, cout 128]
    wq = np.zeros((NP, NT, 3, 128, 128), f32)
    bq = np.zeros((128, NP * 3), f32)
    for j in range(NP):
        for ty in range(3):
            rows = qkv_w[ty * 512 + j * 128 : ty * 512 + (j + 1) * 128]  # [128, C]
            for kt in range(NT):
                wq[j, kt, ty] = rows[:, kt * 128 : (kt + 1) * 128].T
            bq[:, j * 3 + ty] = qkv_b[ty * 512 + j * 128 : ty * 512 + (j + 1) * 128]
    wp = np.ascontiguousarray(
        proj_w.T.reshape(NT, 128, C)
    )  # [ktile, cin 128, cout C]

    bproj = np.ascontiguousarray(proj_b.reshape(NT, 128).T)
    gnw = np.ascontiguousarray(gn_w.reshape(NT, 128).T)
    gnb = np.ascontiguousarray(gn_b.reshape(NT, 128).T)

    p = np.arange(128)
    gmask = np.zeros((128, 8), f32)
    gmask[p, p // 16] = 1.0 / 16.0
    gmaskT = np.ascontiguousarray(
        (np.arange(128)[:, None] // 16 == np.arange(8)[None, :]).astype(f32).T
    )
    ident2 = np.ascontiguousarray(np.eye(128, dtype=f32))

    smalls = np.concatenate([bq, bproj, gnw, gnb, gmask], axis=1)
    assert smalls.shape == (128, 32)

    common = dict(
        wqkvT=wq.astype(bf), wprojT=wp.astype(bf),
        smalls=np.ascontiguousarray(smalls), gmaskT=gmaskT,
        ident2=ident2.astype(bf),
    )
    return [
        dict(common, x=np.ascontiguousarray(x[b]).astype(bf)) for b in range(B)
    ]


def _run(in_maps, trace=False, **kw):
    from concourse.bass_utils import run_bass_kernel_spmd

    nc = _get_nc()
    return run_bass_kernel_spmd(nc, in_maps, list(range(NCORES)), trace=trace, **kw)


def kernel(x, gn_w, gn_b, qkv_w, qkv_b, proj_w, proj_b):
    in_maps = _host_inputs(x, gn_w, gn_b, qkv_w, qkv_b, proj_w, proj_b)
    res = _run(in_maps)
    out = np.stack(
        [np.asarray(res.results[b]["out"]).astype(np.float32) for b in range(B)]
    )
    return out.reshape(B, C, HH, WW)


# revision 27
# speedup vs baseline: 1.1802x; 1.1802x over previous
"""AttentionBlock (GroupNorm -> QKV -> 8-head attention -> proj -> residual)
as a Bass/Tile kernel for Trainium2, data-parallel over batch on 8 cores.

Self-contained: hardcodes shapes B=8, C=512, H=W=32 (N=1024), heads=8, d=64,
groups=32.  Each core processes one batch element; all params replicated.

v2 design (engine-overlap rewrite of the serial-phase baseline):
  * all-bf16 dataflow: x, weights, activations are bf16 (PE streams 1
    cycle/col for any free size; DMA traffic halves; f32 only in PSUM
    accumulators and GN statistics).
  * pair-blocked QKV: weights laid out per head-pair (q|k|v m-tiles of 128
    rows = 2 heads x 64); score matmuls contract over K=64 partition rows
    directly (no zero-padded K copy), saving 4 of 16 QKV m-tiles.
  * softmax denominators via the ones-block trick (vT cols 64-127 are ones so
    context rows 64-127 accumulate sum(probs) for free), then DVE
    reciprocal_approx_fast + multiply -- the ACT engine runs ONLY exp (one
    table load), never switching LUTs mid-kernel.
  * software pipeline: pair j's score matmuls / exp / context accumulation
    run with QKV + v-transposes of pair j+1 interleaved in the PE stream as
    filler, so the PE never waits on the ACT exp backlog and the HAM
    clock-gate stays warm (PE p-state ramps 0.65->2.4 GHz over ~3us).
  * context accumulated per query-half ([128,512] PSUM tiles) so the PSUM
    budget fits exactly 8 banks: scores 2x2 + context 2x1 + qkv/transpose
    staging 2x1.  Pair 3 (no qkv filler left) interleaves its second-half
    context into the kt loop, borrowing the idle staging banks.
  * v-transposes batched: 8 PE transposes land in one [128,512] bf16 PSUM
    tile, evacuated by a single strided DVE copy into persistent vT tiles
    whose ones-columns are memset once.
"""

import sys

sys.path.insert(0, "/opt/trn_rl_repo")

import numpy as np
import ml_dtypes

B, C, HH, WW = 8, 512, 32, 32
N = HH * WW          # 1024
NH, HD = 8, 64       # heads, head dim
NG = 32              # groupnorm groups
EPS = 1e-5
NT = C // 128        # 4 channel tiles
KT = N // 128        # 8 key tiles
NP = NH // 2         # 4 head pairs
NCORES = 8
LAG = 2

_CACHE: dict = {}
DEBUG = False


def _build_program():
    import concourse.bacc as bacc
    import concourse.tile as tile
    from concourse import mybir

    f32 = mybir.dt.float32
    bf16 = mybir.dt.bfloat16
    AF = mybir.ActivationFunctionType
    OP = mybir.AluOpType

    nc = bacc.Bacc("TRN2", target_bir_lowering=False, debug=False)

    x_d = nc.dram_tensor("x", [C, N], bf16, kind="ExternalInput").ap()
    # pair-blocked qkv weights: [pair, ktile, q|k|v, cin 128, cout 128]
    wq_d = nc.dram_tensor("wqkvT", [NP, NT, 3, 128, 128], bf16,
                          kind="ExternalInput").ap()
    wp_d = nc.dram_tensor("wprojT", [NT, 128, C], bf16, kind="ExternalInput").ap()
    smalls_d = nc.dram_tensor("smalls", [128, 32], f32, kind="ExternalInput").ap()
    gmaskT_d = nc.dram_tensor("gmaskT", [8, 128], f32, kind="ExternalInput").ap()
    ident_d = nc.dram_tensor("ident2", [128, 128], bf16, kind="ExternalInput").ap()
    out_d = nc.dram_tensor("out", [C, N], bf16, kind="ExternalOutput").ap()

    x_dt = x_d.rearrange("(t p) n -> t p n", p=128)
    out_dt = out_d.rearrange("(t p) n -> t p n", p=128)

    from contextlib import ExitStack

    with tile.TileContext(nc) as tc, ExitStack() as ctx:
        sg = ctx.enter_context(tc.tile_pool(name="sg", bufs=1))
        work = ctx.enter_context(tc.tile_pool(name="work", bufs=1))
        pb_pool = ctx.enter_context(tc.tile_pool(name="pbp", bufs=2))
        small = ctx.enter_context(tc.tile_pool(name="small", bufs=4))
        outp = ctx.enter_context(tc.tile_pool(name="outp", bufs=2))
        rsp = ctx.enter_context(tc.tile_pool(name="rsp", bufs=2))
        # PSUM (8 banks): pA = scores 2x[128,1024]f32 (2 banks each),
        # pX = context accumulators 2x[128,512]f32, pC = staging 2x[128,512]
        # (qkv halves / transpose batches / GN / proj; pair 3 borrows it for
        # its second-half context).
        pA = ctx.enter_context(tc.tile_pool(name="pA", bufs=2, space="PSUM"))
        pX = ctx.enter_context(tc.tile_pool(name="pX", bufs=2, space="PSUM"))
        pC = ctx.enter_context(tc.tile_pool(name="pC", bufs=2, space="PSUM"))

        # ---- input DMAs ----
        x_sb = []
        for t in range(NT):
            xt = work.tile([128, N], bf16, name=f"x{t}", tag=f"x{t}")
            nc.sync.dma_start(out=xt[:, 0:512], in_=x_dt[t][:, 0:512])
            nc.sync.dma_start(out=xt[:, 512:1024], in_=x_dt[t][:, 512:1024])
            x_sb.append(xt)

        # pair-0 qkv weights first (first consumer), then packed small
        # params, then the remaining weight blocks.
        w_sb = []
        for j in range(NP):
            w_sb.append(
                sg.tile([128, NT * 3 * 128], bf16, name=f"w{j}", tag=f"w{j}")
            )

        def dma_wpair(j):
            nc.sync.dma_start(
                out=w_sb[j].rearrange("p (b c) -> p b c", c=128),
                in_=wq_d[j].rearrange("k t p c -> p (k t) c"),
            )

        dma_wpair(0)
        smalls_sb = sg.tile([128, 32], f32, name="smalls_sb")
        nc.sync.dma_start(out=smalls_sb, in_=smalls_d)
        bqkv_sb = smalls_sb[:, 0:12]
        bproj_sb = smalls_sb[:, 12:16]
        gnw_sb = smalls_sb[:, 16:20]
        gnb_sb = smalls_sb[:, 20:24]
        gmask_sb = smalls_sb[:, 24:32]
        gmaskT_sb = sg.tile([8, 128], f32, name="gmaskT_sb")
        nc.sync.dma_start(out=gmaskT_sb, in_=gmaskT_d)
        ident_sb = sg.tile([128, 128], bf16, name="ident_sb")
        nc.sync.dma_start(out=ident_sb, in_=ident_d)
        for j in range(1, NP):
            dma_wpair(j)
        wp_all = sg.tile([128, NT * C], bf16, name="wp_all")
        nc.sync.dma_start(
            out=wp_all.rearrange("p (t c) -> p t c", c=C),
            in_=wp_d.rearrange("t p c -> p t c"),
        )
        wp_sb = [wp_all[:, t * C : (t + 1) * C] for t in range(NT)]

        eps_sb = sg.tile([8, 1], f32, name="eps_sb")
        nc.gpsimd.memset(eps_sb, EPS)
        # dummy activation: pulls the Sqrt ACT-table load to t~7us (the
        # implicit load otherwise inherits the real sqrt's waits and lands
        # on the GroupNorm critical path)
        sqrt_warm = sg.tile([8, 1], f32, name="sqrt_warm")
        nc.scalar.activation(out=sqrt_warm, in_=eps_sb, func=AF.Sqrt, scale=1.0)

        # persistent vT tiles (per kt a [64 ones | 64 v] block; ones memset
        # once) and zero-padded per-head K tiles (K=64-contraction matmuls
        # misbehave on HW; keep full-128 contraction with zeros in the other
        # head's rows).  Memsets run on the idle GPSIMD engine, ordered so
        # pair 0's tiles (first consumers) are ready earliest.
        vt_sb = [[None, None] for _ in range(NP)]
        kp_sb = [[None, None] for _ in range(NP)]
        for j in range(NP):
            for h01 in range(2):
                kp = sg.tile([128, N], bf16, name=f"kp{j}_{h01}")
                po = (1 - h01) * HD
                nc.gpsimd.memset(kp[po : po + HD, :], 0.0)
                kp_sb[j][h01] = kp
            for h01 in range(2):
                vt = sg.tile([128, N], bf16, name=f"vt{j}_{h01}")
                nc.gpsimd.memset(
                    vt.rearrange("p (k c) -> p k c", c=128)[:, :, 0:HD], 1.0
                )
                vt_sb[j][h01] = vt

        # ---- GroupNorm statistics ----
        allstats = sg.tile([128, 2 * NT], f32, name="allstats")
        for t in range(NT):
            bns = small.tile([128, 2, 6], f32, name=f"bns{t}", tag="bns")
            nc.vector.bn_stats(out=bns[:, 0, :], in_=x_sb[t][:, 0:512])
            nc.vector.bn_stats(out=bns[:, 1, :], in_=x_sb[t][:, 512:1024])
            nc.vector.bn_aggr(out=allstats[:, 2 * t : 2 * t + 2], in_=bns)
            m2 = small.tile([128, 1], f32, name=f"m2_{t}", tag="m2")
            nc.vector.tensor_mul(
                m2, allstats[:, 2 * t : 2 * t + 1], allstats[:, 2 * t : 2 * t + 1]
            )
            nc.vector.tensor_add(
                allstats[:, 2 * t + 1 : 2 * t + 2],
                allstats[:, 2 * t + 1 : 2 * t + 2],
                m2,
            )

        grp_ps = pC.tile([8, 2 * NT], f32, name="grp_ps", tag="acc")
        nc.tensor.matmul(grp_ps, gmask_sb, allstats)
        grp_sb = sg.tile([8, 2 * NT], f32, name="grp_sb")
        nc.vector.tensor_copy(grp_sb, grp_ps)
        msq = sg.tile([8, NT], f32, name="msq")
        nc.vector.tensor_mul(msq, grp_sb[:, 0 : 2 * NT : 2], grp_sb[:, 0 : 2 * NT : 2])
        nc.vector.tensor_sub(
            grp_sb[:, 1 : 2 * NT : 2], grp_sb[:, 1 : 2 * NT : 2], msq
        )
        nc.scalar.activation(
            out=grp_sb[:, 1 : 2 * NT : 2],
            in_=grp_sb[:, 1 : 2 * NT : 2],
            func=AF.Sqrt,
            bias=eps_sb,
            scale=1.0,
        )
        nc.vector.reciprocal(grp_sb[:, 1 : 2 * NT : 2], grp_sb[:, 1 : 2 * NT : 2])

        chan_ps = pC.tile([128, 2 * NT], f32, name="chan_ps", tag="acc")
        nc.tensor.matmul(chan_ps, gmaskT_sb, grp_sb)
        chan_sb = sg.tile([128, 2 * NT], f32, name="chan_sb")
        nc.vector.tensor_copy(chan_sb, chan_ps)

        A_sb = sg.tile([128, NT], f32, name="A_sb")
        nc.vector.tensor_mul(A_sb, chan_sb[:, 1 : 2 * NT : 2], gnw_sb)
        B_sb = sg.tile([128, NT], f32, name="B_sb")
        nc.vector.tensor_mul(B_sb, chan_sb[:, 0 : 2 * NT : 2], A_sb)
        nc.vector.tensor_sub(B_sb, gnb_sb, B_sb)

        xn_sb = []
        for t in range(NT):
            xn_sb.append(work.tile([128, N], bf16, name=f"xn{t}", tag=f"xn{t}"))
        for hlf in range(2):
            for t in range(NT):
                nc.vector.tensor_scalar(
                    out=xn_sb[t][:, hlf * 512 : (hlf + 1) * 512],
                    in0=x_sb[t][:, hlf * 512 : (hlf + 1) * 512],
                    scalar1=A_sb[:, t : t + 1],
                    scalar2=B_sb[:, t : t + 1],
                    op0=OP.mult,
                    op1=OP.add,
                )

        # ---- QKV / transpose building blocks ----
        q_sb = [None] * NP
        k_sb = [None] * NP
        v_sb = [None] * NP

        def qkv_half(j, ty, hlf):
            """One m-half of pair j's q/k/v projection (4 matmuls + evac)."""
            dest = (q_sb, k_sb, v_sb)[ty]
            if dest[j] is None:
                dest[j] = work.tile(
                    [128, N], bf16, name=f"qkv{j}_{ty}", tag=f"qkv{j}_{ty}"
                )
            acc = pC.tile([128, 512], f32, name=f"acc{j}_{ty}_{hlf}", tag="acc")
            for kc in range(NT):
                nc.tensor.matmul(
                    acc,
                    w_sb[j][:, (kc * 3 + ty) * 128 : (kc * 3 + ty + 1) * 128],
                    xn_sb[kc][:, hlf * 512 : (hlf + 1) * 512],
                    start=(kc == 0),
                    stop=(kc == NT - 1),
                )
            nc.vector.tensor_scalar_add(
                dest[j][:, hlf * 512 : (hlf + 1) * 512],
                acc,
                bqkv_sb[:, j * 3 + ty : j * 3 + ty + 1],
            )
            if ty == 1:
                # stage this K half into the zero-padded per-head tiles
                for h01 in range(2):
                    po = h01 * HD
                    nc.vector.tensor_copy(
                        kp_sb[j][h01][po : po + HD, hlf * 512 : (hlf + 1) * 512],
                        k_sb[j][po : po + HD, hlf * 512 : (hlf + 1) * 512],
                    )

        def vtrans_pair(j):
            """All 8 key-chunks of BOTH heads' v transposed in one go
            ([128,128] blocks), then per-head strided DVE copies into the
            persistent vT tiles' v-slots."""
            tp = pC.tile([128, N], bf16, name=f"tp{j}", tag="acc")
            for kt in range(KT):
                nc.tensor.transpose(
                    tp[:, kt * 128 : (kt + 1) * 128],
                    v_sb[j][:, kt * 128 : (kt + 1) * 128],
                    ident_sb,
                )
            for h01 in range(2):
                nc.vector.tensor_copy(
                    vt_sb[j][h01].rearrange("p (k c) -> p k c", c=128)[:, :, HD:128],
                    tp.rearrange("p (k c) -> p k c", c=128)[:, :, h01 * HD : (h01 + 1) * HD],
                )

        def qkv_pair(j):
            for ty in range(3):
                for hlf in range(2):
                    qkv_half(j, ty, hlf)

        hatt_sb = []
        for t in range(NT):
            ht = work.tile([128, N], bf16, name=f"hatt{t}", tag=f"hatt{t}")
            hatt_sb.append(ht)

        dbg_tiles = {}
        if DEBUG:
            dbg_tiles["pb00"] = sg.tile([128, N], bf16, name="dbg_t_pb00")
            dbg_tiles["cx10"] = sg.tile([128, 512], f32, name="dbg_t_cx10")
            dbg_tiles["rs00"] = sg.tile([HD, 512], f32, name="dbg_t_rs00")

        # ---- attention pair with interleaved filler ----
        def attn_pair(j):
            last = j == NP - 1
            pbs = {}
            cx1 = {}
            cx2 = {}
            # filler units for pairs 0-2: qkv halves + v transposes of j+1
            filler = []
            if not last:
                jn = j + 1
                for ty in range(3):
                    for hlf in range(2):
                        filler.append(lambda ty=ty, hlf=hlf: qkv_half(jn, ty, hlf))
                filler.append(lambda: vtrans_pair(jn))

            def emit_sc(kt):
                for h01 in range(2):
                    sc = pA.tile([128, N], f32, name=f"sc{h01}_{kt}", tag="sc")
                    for hlf in range(2):
                        nc.tensor.matmul(
                            sc[:, hlf * 512 : (hlf + 1) * 512],
                            kp_sb[j][h01][:, kt * 128 : (kt + 1) * 128],
                            q_sb[j][:, hlf * 512 : (hlf + 1) * 512],
                        )
                    pb = pb_pool.tile(
                        [128, N], bf16, name=f"pb{h01}_{kt}", tag=f"pb{h01}_{kt}"
                    )
                    nc.scalar.activation(out=pb, in_=sc, func=AF.Exp, scale=0.125)
                    pbs[(h01, kt)] = pb
                    if DEBUG and j == 0 and h01 == 0 and kt == 0:
                        nc.vector.tensor_copy(dbg_tiles["pb00"], pb)

            def emit_cx1(kt):
                for h01 in range(2):
                    if kt == 0:
                        cx1[h01] = pX.tile(
                            [128, 512], f32, name=f"cx1_{h01}", tag="cx"
                        )
                    nc.tensor.matmul(
                        cx1[h01],
                        vt_sb[j][h01][:, kt * 128 : (kt + 1) * 128],
                        pbs[(h01, kt)][:, 0:512],
                        start=(kt == 0),
                        stop=(kt == KT - 1),
                    )

            def emit_cx2(kt):
                for h01 in range(2):
                    if kt == 0:
                        pool, tag = (pC, "acc") if last else (pX, "cx")
                        cx2[h01] = pool.tile(
                            [128, 512], f32, name=f"cx2_{h01}", tag=tag
                        )
                    nc.tensor.matmul(
                        cx2[h01],
                        vt_sb[j][h01][:, kt * 128 : (kt + 1) * 128],
                        pbs[(h01, kt)][:, 512:1024],
                        start=(kt == 0),
                        stop=(kt == KT - 1),
                    )

            def normalize(cx, hlf):
                for h01 in range(2):
                    rsb = rsp.tile([HD, 512], f32, name=f"rs{h01}", tag="rs")
                    if DEBUG and j == 0 and h01 == 0 and hlf == 0:
                        nc.vector.tensor_copy(dbg_tiles["cx10"], cx[h01])
                    nc.vector.reciprocal_approx_fast(out=rsb, in_=cx[h01][0:HD, :])
                    if DEBUG and j == 0 and h01 == 0 and hlf == 0:
                        nc.vector.tensor_copy(dbg_tiles["rs00"], rsb)
                    nc.vector.tensor_mul(
                        hatt_sb[j][h01 * HD : (h01 + 1) * HD,
                                   hlf * 512 : (hlf + 1) * 512],
                        cx[h01][HD:128, :],
                        rsb,
                    )

            # kt loop: scores + exp lead; first-half context LAGs; filler
            # (next pair's qkv) keeps the PE fed while ACT drains the exp
            # backlog.  Pair 3 interleaves second-half context instead.
            fi = 0
            for kt in range(KT):
                emit_sc(kt)
                if kt >= LAG:
                    emit_cx1(kt - LAG)
                    if last:
                        emit_cx2(kt - LAG)
                if kt > 0 and fi < len(filler):
                    filler[fi]()
                    fi += 1
            for kt in range(KT - LAG, KT):
                emit_cx1(kt)
                if last:
                    emit_cx2(kt)
            while fi < len(filler):
                filler[fi]()
                fi += 1
            normalize(cx1, 0)
            if not last:
                for kt in range(KT):
                    emit_cx2(kt)
            normalize(cx2, 1)

        qkv_pair(0)
        vtrans_pair(0)
        for j in range(NP):
            attn_pair(j)

        # ---- proj + bias + residual ----
        for mt in range(NT):
            ot = outp.tile([128, N], bf16, name=f"ot{mt}", tag="ot")
            for hlf in range(2):
                # alternate pX/pC so four accumulators are in flight and
                # the STT evacuations never stall the PE (pX frees after
                # pair 3's first-half normalize; pC after its second).
                ppool, ptag = (pX, "cx") if hlf == 0 else (pC, "acc")
                pp = ppool.tile([128, 512], f32, name=f"pp{mt}_{hlf}", tag=ptag)
                for kc in range(NT):
                    nc.tensor.matmul(
                        pp,
                        wp_sb[kc][:, mt * 128 : (mt + 1) * 128],
                        hatt_sb[kc][:, hlf * 512 : (hlf + 1) * 512],
                        start=(kc == 0),
                        stop=(kc == NT - 1),
                    )
                nc.vector.scalar_tensor_tensor(
                    out=ot[:, hlf * 512 : (hlf + 1) * 512],
                    in0=pp,
                    scalar=bproj_sb[:, mt : mt + 1],
                    in1=x_sb[mt][:, hlf * 512 : (hlf + 1) * 512],
                    op0=OP.add,
                    op1=OP.add,
                )
                nc.sync.dma_start(
                    out=out_dt[mt][:, hlf * 512 : (hlf + 1) * 512],
                    in_=ot[:, hlf * 512 : (hlf + 1) * 512],
                )

        if DEBUG:
            dbg_specs = [
                ("dbg_w0", w_sb[0], [128, NT * 3 * 128], bf16),
                ("dbg_xn0", xn_sb[0], [128, N], bf16),
                ("dbg_q0", q_sb[0], [128, N], bf16),
                ("dbg_k0", k_sb[0], [128, N], bf16),
                ("dbg_kp00", kp_sb[0][0], [128, N], bf16),
                ("dbg_v0", v_sb[0], [128, N], bf16),
                ("dbg_vt00", vt_sb[0][0], [128, N], bf16),
                ("dbg_hatt0", hatt_sb[0], [128, N], bf16),
                ("dbg_A", A_sb, [128, NT], f32),
                ("dbg_pb00", dbg_tiles["pb00"], [128, N], bf16),
                ("dbg_cx10", dbg_tiles["cx10"], [128, 512], f32),
                ("dbg_rs00", dbg_tiles["rs00"], [HD, 512], f32),
            ]
            for nm, tile_, shp, dt_ in dbg_specs:
                d = nc.dram_tensor(nm, shp, dt_, kind="ExternalOutput").ap()
                nc.sync.dma_start(out=d, in_=tile_)

    nc.compile()
    return nc


def _get_nc():
    if "nc" not in _CACHE:
        _CACHE["nc"] = _build_program()
    return _CACHE["nc"]


def _host_inputs(x, gn_w, gn_b, qkv_w, qkv_b, proj_w, proj_b):
    f32 = np.float32
    bf = ml_dtypes.bfloat16
    x = np.asarray(x, dtype=f32).reshape(B, C, N)
    gn_w = np.asarray(gn_w, dtype=f32)
    gn_b = np.asarray(gn_b, dtype=f32)
    qkv_w = np.asarray(qkv_w, dtype=f32)
    qkv_b = np.asarray(qkv_b, dtype=f32)
    proj_w = np.asarray(proj_w, dtype=f32)
    proj_b = np.asarray(proj_b, dtype=f32)

    # pair-blocked qkv weights: [pair, ktile, q|k|v, cin 128, cout 128]
    wq = np.zeros((NP, NT, 3, 128, 128), f32)
    bq = np.zeros((128, NP * 3), f32)
    for j in range(NP):
        for ty in range(3):
            rows = qkv_w[ty * 512 + j * 128 : ty * 512 + (j + 1) * 128]  # [128, C]
            for kt in range(NT):
                wq[j, kt, ty] = rows[:, kt * 128 : (kt + 1) * 128].T
            bq[:, j * 3 + ty] = qkv_b[ty * 512 + j * 128 : ty * 512 + (j + 1) * 128]
    wp = np.ascontiguousarray(
        proj_w.T.reshape(NT, 128, C)
    )  # [ktile, cin 128, cout C]

    bproj = np.ascontiguousarray(proj_b.reshape(NT, 128).T)
    gnw = np.ascontiguousarray(gn_w.reshape(NT, 128).T)
    gnb = np.ascontiguousarray(gn_b.reshape(NT, 128).T)

    p = np.arange(128)
    gmask = np.zeros((128, 8), f32)
    gmask[p, p // 16] = 1.0 / 16.0
    gmaskT = np.ascontiguousarray(
        (np.arange(128)[:, None] // 16 == np.arange(8)[None, :]).astype(f32).T
    )
    ident2 = np.ascontiguousarray(np.eye(128, dtype=f32))

    smalls = np.concatenate([bq, bproj, gnw, gnb, gmask], axis=1)
    assert smalls.shape == (128, 32)

    common = dict(
        wqkvT=wq.astype(bf), wprojT=wp.astype(bf),
        smalls=np.ascontiguousarray(smalls), gmaskT=gmaskT,
        ident2=ident2.astype(bf),
    )
    return [
        dict(common, x=np.ascontiguousarray(x[b]).astype(bf)) for b in range(B)
    ]


def _run(in_maps, trace=False, **kw):
    from concourse.bass_utils import run_bass_kernel_spmd

    nc = _get_nc()
    return run_bass_kernel_spmd(nc, in_maps, list(range(NCORES)), trace=trace, **kw)


def kernel(x, gn_w, gn_b, qkv_w, qkv_b, proj_w, proj_b):
    in_maps = _host_inputs(x, gn_w, gn_b, qkv_w, qkv_b, proj_w, proj_b)
    res = _run(in_maps)
    out = np.stack(
        [np.asarray(res.results[b]["out"]).astype(np.float32) for b in range(B)]
    )
    return out.reshape(B, C, HH, WW)
